# revision 4
# baseline (speedup 1.0000x reference)
"""Trainium2 Bass kernel for 16-head causal attention with relative position
bias (B=4, S=2048, D=1024, H=16, HD=64), distributed over 8 NeuronCores.

Sharding: tensor-parallel over heads — each core owns 2 heads end-to-end
(QKV projection column-sharded, attention, then an on-device AllToAll
re-shards by tokens so each core runs the output projection for a disjoint
1024-token slice). Host only slices weights / concatenates output slices.

Host-side prep folds work into the data layout:
  - x is passed transposed ([D, B*S]) so it can feed matmuls directly as rhs.
  - HD^-0.5 scaling folded into Wq/bq.
  - causal mask folded into rel_bias (bias_c = rel_bias[heads] + causal).
  - key-padding additive mask pre-broadcast to 128 partitions.
Softmax is computed without max-subtraction (scores are O(6), exp is safe in
fp32) and the 1/rowsum normalization is applied to P before P@V.
"""

import numpy as np

import concourse.bass as bass
import concourse.mybir as mybir
from concourse import bacc
from concourse.tile import TileContext
from concourse.masks import make_identity
from concourse.bass_utils import run_bass_kernel_spmd

B, S, D, H = 4, 2048, 1024, 16
HD = D // H                  # 64
NC_ = 8                      # cores
HPC = H // NC_               # 2 heads per core
T = B * S                    # 8192 tokens
TPC = T // NC_               # 1024 tokens per core (out-proj shard)
NEG = -1e9
FP32 = mybir.dt.float32

# number of 512-wide k-blocks (= q macro blocks) per sequence
NKB = S // 512               # 4
QT_TILES = S // 128          # 16 q tiles per sequence


def build_program(first_pad_kj: int) -> bass.Bass:
    """Build the (identical-on-every-core) SPMD Bass program.

    first_pad_kj: first 512-wide k-block index that can contain padded keys
    (padding additive mask is only applied for kj >= first_pad_kj).
    """
    nc = bacc.Bacc(num_devices=NC_)

    # ---- I/O ----
    xT = nc.dram_tensor("xT", [D, T], FP32, kind="ExternalInput")
    wq = nc.dram_tensor("wq", [D, 128], FP32, kind="ExternalInput")
    wk = nc.dram_tensor("wk", [D, 128], FP32, kind="ExternalInput")
    wv = nc.dram_tensor("wv", [D, 128], FP32, kind="ExternalInput")
    bq = nc.dram_tensor("bq", [128], FP32, kind="ExternalInput")
    bk = nc.dram_tensor("bk", [128], FP32, kind="ExternalInput")
    bvb = nc.dram_tensor("bvb", [128, 128], FP32, kind="ExternalInput")
    biasc = nc.dram_tensor("biasc", [HPC, S, S], FP32, kind="ExternalInput")
    pb = nc.dram_tensor("pb", [128, B, S], FP32, kind="ExternalInput")
    wout = nc.dram_tensor("wout", [D, D], FP32, kind="ExternalInput")
    boutb = nc.dram_tensor("boutb", [128, D], FP32, kind="ExternalInput")
    npad = nc.dram_tensor("npad", [128, TPC // 128], FP32, kind="ExternalInput")
    out = nc.dram_tensor("out", [TPC, D], FP32, kind="ExternalOutput")

    npads = TPC // 128  # 8 token tiles in out-proj

    with TileContext(nc) as tc:
        with tc.tile_pool(name="const", bufs=1) as const, \
             tc.tile_pool(name="big", bufs=1) as big:
            # ---- constants ----
            ident = const.tile([128, 128], FP32, tag="ident")
            make_identity(nc, ident)
            wq_sb = const.tile([128, 8, 128], FP32, tag="wq")
            wk_sb = const.tile([128, 8, 128], FP32, tag="wk")
            wv_sb = const.tile([128, 8, 128], FP32, tag="wv")
            nc.sync.dma_start(wq_sb, wq.rearrange("(fo p) m -> p fo m", p=128))
            nc.sync.dma_start(wk_sb, wk.rearrange("(fo p) m -> p fo m", p=128))
            nc.sync.dma_start(wv_sb, wv.rearrange("(fo p) m -> p fo m", p=128))
            bq_sb = const.tile([128, 1], FP32, tag="bq")
            bk_sb = const.tile([128, 1], FP32, tag="bk")
            nc.sync.dma_start(bq_sb, bq[:, None])
            nc.sync.dma_start(bk_sb, bk[:, None])
            bvb_sb = const.tile([128, 128], FP32, tag="bvb")
            nc.sync.dma_start(bvb_sb, bvb[:])
            npb = NKB - first_pad_kj  # how many 512-blocks of padding bias
            if npb > 0:
                pb_sb = const.tile([128, B, npb * 512], FP32, tag="pb")
                nc.sync.dma_start(pb_sb, pb[:, :, first_pad_kj * 512:])

            # ---- persistent per-core intermediates ----
            # QT/KT: [2*HD qdims (h0 rows 0:64, h1 rows 64:128), B*S tokens]
            QT = big.tile([128, T], FP32, tag="QT")
            KT = big.tile([128, T], FP32, tag="KT")
            # V: [128 token-part, 64 token-chunks, 128 vdims(2 heads)]
            V = big.tile([128, T // 128, 128], FP32, tag="V")

            # ---- internal DRAM for the AllToAll ----
            with tc.tile_pool(name="dram", bufs=1, space="DRAM") as dpool:
                a2a_in = dpool.tile([NC_, 128, TPC], FP32, tag="a2a_in")
                a2a_out = dpool.tile([NC_, 128, TPC], FP32, tag="a2a_out")

                # ================= Phase B: QKV projection =================
                xT_r = xT.rearrange("(fo p) t -> p fo t", p=128)
                with tc.tile_pool(name="qkv", bufs=3) as qkvp, \
                     tc.tile_pool(name="qkv_ps", bufs=2, space="PSUM") as qps:
                    for tb in range(T // 512):
                        xt = qkvp.tile([128, 8, 512], FP32, tag="xt")
                        nc.sync.dma_start(xt, xT_r[:, :, tb * 512:(tb + 1) * 512])
                        psq = qps.tile([128, 512], FP32, tag="psq")
                        psk = qps.tile([128, 512], FP32, tag="psk")
                        for fo in range(8):
                            nc.tensor.matmul(psq, wq_sb[:, fo], xt[:, fo],
                                             start=(fo == 0), stop=(fo == 7))
                        for fo in range(8):
                            nc.tensor.matmul(psk, wk_sb[:, fo], xt[:, fo],
                                             start=(fo == 0), stop=(fo == 7))
                        sl = slice(tb * 512, (tb + 1) * 512)
                        nc.scalar.activation(
                            QT[:, sl], psq,
                            mybir.ActivationFunctionType.Identity, bias=bq_sb)
                        nc.scalar.activation(
                            KT[:, sl], psk,
                            mybir.ActivationFunctionType.Identity, bias=bk_sb)
                        for t4 in range(4):
                            psv = qps.tile([128, 128], FP32, tag="psv")
                            for fo in range(8):
                                nc.tensor.matmul(
                                    psv, xt[:, fo, t4 * 128:(t4 + 1) * 128],
                                    wv_sb[:, fo],
                                    start=(fo == 0), stop=(fo == 7))
                            nc.vector.tensor_add(
                                out=V[:, tb * 4 + t4, :], in0=psv, in1=bvb_sb)

                # ================= Phase C: attention =================
                with tc.tile_pool(name="att", bufs=2) as att, \
                     tc.tile_pool(name="attb", bufs=2) as attb, \
                     tc.tile_pool(name="P", bufs=1) as Pp, \
                     tc.tile_pool(name="pt", bufs=3) as ptp, \
                     tc.tile_pool(name="att_ps", bufs=2, space="PSUM") as aps:
                    for h in range(HPC):
                        hsl = slice(h * 64, h * 64 + 64)
                        for b in range(B):
                            for qmb in range(NKB):
                                nkb = qmb + 1
                                kw = nkb * 512  # key width
                                Pb = Pp.tile([128, 4, NKB * 512], FP32, tag="P")
                                for qi in range(4):
                                    qt = qmb * 4 + qi
                                    bias_t = attb.tile([128, NKB * 512], FP32,
                                                       tag="bias")
                                    nc.sync.dma_start(
                                        bias_t[:, :kw],
                                        biasc[h, qt * 128:(qt + 1) * 128, :kw])
                                    s_sb = attb.tile([128, NKB * 512], FP32,
                                                     tag="s")
                                    for kj in range(nkb):
                                        ps = aps.tile([128, 512], FP32, tag="s_ps")
                                        nc.tensor.matmul(
                                            ps,
                                            QT[hsl, b * S + qt * 128:
                                               b * S + (qt + 1) * 128],
                                            KT[hsl, b * S + kj * 512:
                                               b * S + (kj + 1) * 512],
                                            start=True, stop=True)
                                        ksl = slice(kj * 512, (kj + 1) * 512)
                                        nc.vector.tensor_add(
                                            out=s_sb[:, ksl], in0=ps,
                                            in1=bias_t[:, ksl])
                                        if kj >= first_pad_kj:
                                            nc.vector.tensor_add(
                                                out=s_sb[:, ksl],
                                                in0=s_sb[:, ksl],
                                                in1=pb_sb[:, b,
                                                          (kj - first_pad_kj) * 512:
                                                          (kj - first_pad_kj + 1) * 512])
                                    rs = att.tile([128, 1], FP32, tag="rs")
                                    nc.scalar.activation(
                                        Pb[:, qi, :kw], s_sb[:, :kw],
                                        mybir.ActivationFunctionType.Exp,
                                        accum_out=rs)
                                    rc = att.tile([128, 1], FP32, tag="rc")
                                    nc.vector.reciprocal(rc, rs)
                                    nc.vector.tensor_scalar_mul(
                                        Pb[:, qi, :kw], Pb[:, qi, :kw], rc)
                                # transpose P and accumulate P^T into attn@V
                                av_ps = aps.tile([64, 512], FP32, tag="av")
                                for kc in range(nkb * 4):
                                    pt_ps = aps.tile([128, 512], FP32, tag="ptps")
                                    for qi in range(4):
                                        nc.tensor.transpose(
                                            pt_ps[:, qi * 128:(qi + 1) * 128],
                                            Pb[:, qi, kc * 128:(kc + 1) * 128],
                                            ident)
                                    pt_sb = ptp.tile([128, 512], FP32, tag="pt")
                                    nc.vector.tensor_copy(out=pt_sb, in_=pt_ps)
                                    nc.tensor.matmul(
                                        av_ps, V[:, b * 16 + kc, hsl], pt_sb,
                                        start=(kc == 0), stop=(kc == nkb * 4 - 1))
                                av_sb = att.tile([64, 512], FP32, tag="avsb")
                                nc.vector.tensor_copy(out=av_sb, in_=av_ps)
                                g = b * S + qmb * 512
                                nc.sync.dma_start(
                                    a2a_in[g // TPC, h * 64:h * 64 + 64,
                                           (g % TPC):(g % TPC) + 512],
                                    av_sb)

                # ================= Phase D: AllToAll + out-proj =================
                nc.gpsimd.collective_compute(
                    "AllToAll", mybir.AluOpType.bypass,
                    replica_groups=[list(range(NC_))],
                    ins=[a2a_in[:]], outs=[a2a_out[:]])

                with tc.tile_pool(name="proj", bufs=1) as proj, \
                     tc.tile_pool(name="proj_w", bufs=2) as projw, \
                     tc.tile_pool(name="proj_ps", bufs=2, space="PSUM") as pps:
                    wout_sb = proj.tile([128, 8, D], FP32, tag="wout")
                    nc.sync.dma_start(
                        wout_sb, wout.rearrange("(io p) n -> p io n", p=128))
                    boutb_sb = proj.tile([128, D], FP32, tag="boutb")
                    nc.sync.dma_start(boutb_sb, boutb[:])
                    npad_sb = proj.tile([128, npads], FP32, tag="npad")
                    nc.sync.dma_start(npad_sb, npad[:])
                    recv = []
                    for i in range(NC_):
                        r = proj.tile([128, TPC], FP32, tag=f"recv{i}")
                        nc.sync.dma_start(r, a2a_out[i])
                        recv.append(r)
                    for tt in range(npads):
                        o_sb = projw.tile([128, D], FP32, tag="osb")
                        for nb in range(2):
                            ps = pps.tile([128, 512], FP32, tag="o_ps")
                            for i in range(NC_):
                                nc.tensor.matmul(
                                    ps, recv[i][:, tt * 128:(tt + 1) * 128],
                                    wout_sb[:, i, nb * 512:(nb + 1) * 512],
                                    start=(i == 0), stop=(i == NC_ - 1))
                            nsl = slice(nb * 512, (nb + 1) * 512)
                            nc.vector.tensor_add(
                                out=o_sb[:, nsl], in0=ps, in1=boutb_sb[:, nsl])
                        nc.vector.tensor_scalar_mul(
                            o_sb, o_sb, npad_sb[:, tt:tt + 1])
                        nc.sync.dma_start(
                            out[tt * 128:(tt + 1) * 128, :], o_sb)
    nc.finalize()
    return nc


_CACHE: dict = {}


def _prep_inputs(x, Wqkv, bqkv, Wout, bout, causal_mask, rel_bias,
                 key_padding_mask):
    """Host-side shard prep: returns (in_maps, first_pad_kj)."""
    f32 = np.float32
    x = np.asarray(x, f32)
    Wqkv = np.asarray(Wqkv, f32)
    bqkv = np.asarray(bqkv, f32)
    Wout = np.asarray(Wout, f32)
    bout = np.asarray(bout, f32)
    causal_mask = np.asarray(causal_mask, f32)
    rel_bias = np.asarray(rel_bias, f32)
    kpm = np.asarray(key_padding_mask, bool)

    scale = f32(HD ** -0.5)
    xT = np.ascontiguousarray(x.reshape(T, D).T)

    pad_cols = np.where(kpm.any(axis=0))[0]
    first_pad_kj = int(pad_cols.min() // 512) if pad_cols.size else NKB
    # additive key-padding bias, broadcast to 128 partitions: [128, B, S]
    pb = np.where(kpm, f32(NEG), f32(0.0)).astype(f32)        # [B, S]
    pb = np.ascontiguousarray(np.broadcast_to(pb[None], (128, B, S)))
    boutb = np.ascontiguousarray(np.broadcast_to(bout[None], (128, D)))
    notpad_flat = (~kpm).reshape(T).astype(f32)

    in_maps = []
    for c in range(NC_):
        co = 128 * c
        wq_c = np.ascontiguousarray(Wqkv[:, co:co + 128] * scale)
        wk_c = np.ascontiguousarray(Wqkv[:, D + co:D + co + 128])
        wv_c = np.ascontiguousarray(Wqkv[:, 2 * D + co:2 * D + co + 128])
        bq_c = np.ascontiguousarray(bqkv[co:co + 128] * scale)
        bk_c = np.ascontiguousarray(bqkv[D + co:D + co + 128])
        bv_c = bqkv[2 * D + co:2 * D + co + 128]
        bvb_c = np.ascontiguousarray(np.broadcast_to(bv_c[None], (128, 128)))
        bias_c = rel_bias[HPC * c:HPC * c + HPC] + causal_mask[None]
        np_c = np.ascontiguousarray(
            notpad_flat[c * TPC:(c + 1) * TPC].reshape(TPC // 128, 128).T)
        in_maps.append({
            "xT": xT, "wq": wq_c, "wk": wk_c, "wv": wv_c,
            "bq": bq_c, "bk": bk_c, "bvb": bvb_c,
            "biasc": np.ascontiguousarray(bias_c), "pb": pb,
            "wout": Wout, "boutb": boutb, "npad": np_c,
        })
    return in_maps, first_pad_kj


def kernel(**inputs) -> np.ndarray:
    in_maps, first_pad_kj = _prep_inputs(**inputs)
    key = ("prog", first_pad_kj)
    if key not in _CACHE:
        _CACHE[key] = build_program(first_pad_kj)
    nc = _CACHE[key]
    res = run_bass_kernel_spmd(nc, in_maps, core_ids=list(range(NC_)))
    outs = [res.results[c]["out"] for c in range(NC_)]
    return np.concatenate(outs, axis=0).reshape(B, S, D)


# revision 7
# speedup vs baseline: 1.6495x; 1.6495x over previous
"""Trainium2 Bass kernel for 16-head causal attention with relative position
bias (B=4, S=2048, D=1024, H=16, HD=64), distributed over 8 NeuronCores.

Sharding: tensor-parallel over heads — each core owns 2 heads end-to-end
(QKV projection column-sharded, attention, then an on-device AllToAll
re-shards by tokens so each core runs the output projection for a disjoint
1024-token slice). Host only slices weights / concatenates output slices.

Compute dtype: float32r (full-rate fp32 storage, ~1.5e-4 matmul rel err)
for all matmuls; softmax logits and accumulation stay fp32.

Host-side prep folds work into the data layout:
  - x is passed transposed ([D, B*S]) so it can feed matmuls directly as rhs.
  - HD^-0.5 scaling folded into Wq/bq.
  - causal mask folded into rel_bias (bias_c = rel_bias[heads] + causal),
    shipped as bf16 to halve DMA.
  - key-padding additive mask pre-broadcast to 128 partitions.
Softmax is computed without max-subtraction (scores are O(6), exp is safe in
fp32) and the 1/rowsum normalization is applied to P before P@V.
"""

import numpy as np
import ml_dtypes

import concourse.bass as bass
import concourse.mybir as mybir
from concourse import bacc
from concourse.tile import TileContext
from concourse.masks import make_identity
from concourse.bass_utils import run_bass_kernel_spmd

B, S, D, H = 4, 2048, 1024, 16
HD = D // H                  # 64
NC_ = 8                      # cores
HPC = H // NC_               # 2 heads per core
T = B * S                    # 8192 tokens
TPC = T // NC_               # 1024 tokens per core (out-proj shard)
NEG = -1e9
FP32 = mybir.dt.float32
F32R = mybir.dt.float32r
BF16 = mybir.dt.bfloat16

NKB = S // 512               # 4 k-blocks (and q macro blocks) per sequence
IDENT = mybir.ActivationFunctionType.Identity
EXP = mybir.ActivationFunctionType.Exp


def build_program(first_pad_kj: int) -> bass.Bass:
    """Build the (identical-on-every-core) SPMD Bass program."""
    nc = bacc.Bacc(num_devices=NC_)

    # ---- I/O ----
    xT = nc.dram_tensor("xT", [D, T], F32R, kind="ExternalInput")
    wq = nc.dram_tensor("wq", [D, 128], F32R, kind="ExternalInput")
    wk = nc.dram_tensor("wk", [D, 128], F32R, kind="ExternalInput")
    wv = nc.dram_tensor("wv", [D, 128], F32R, kind="ExternalInput")
    bq = nc.dram_tensor("bq", [128], FP32, kind="ExternalInput")
    bk = nc.dram_tensor("bk", [128], FP32, kind="ExternalInput")
    bv = nc.dram_tensor("bv", [128], FP32, kind="ExternalInput")
    biasc = nc.dram_tensor("biasc", [HPC, S, S], BF16, kind="ExternalInput")
    pb = nc.dram_tensor("pb", [128, B, S], FP32, kind="ExternalInput")
    wout = nc.dram_tensor("wout", [D, D], F32R, kind="ExternalInput")
    boutb = nc.dram_tensor("boutb", [128, D], FP32, kind="ExternalInput")
    npad = nc.dram_tensor("npad", [128, TPC // 128], FP32, kind="ExternalInput")
    out = nc.dram_tensor("out", [TPC, D], FP32, kind="ExternalOutput")

    npads = TPC // 128  # 8 token tiles in out-proj

    with TileContext(nc) as tc:
        with tc.tile_pool(name="const", bufs=1) as const, \
             tc.tile_pool(name="big", bufs=1) as big:
            # ---- constants ----
            ident_f32 = const.tile([128, 128], FP32, tag="ident_f32")
            make_identity(nc, ident_f32)
            ident = const.tile([128, 128], F32R, tag="ident")
            nc.vector.tensor_copy(out=ident, in_=ident_f32)
            wq_sb = const.tile([128, 8, 128], F32R, tag="wq")
            wk_sb = const.tile([128, 8, 128], F32R, tag="wk")
            wv_sb = const.tile([128, 8, 128], F32R, tag="wv")
            nc.sync.dma_start(wq_sb, wq.rearrange("(fo p) m -> p fo m", p=128))
            nc.sync.dma_start(wk_sb, wk.rearrange("(fo p) m -> p fo m", p=128))
            nc.sync.dma_start(wv_sb, wv.rearrange("(fo p) m -> p fo m", p=128))
            bq_sb = const.tile([128, 1], FP32, tag="bq")
            bk_sb = const.tile([128, 1], FP32, tag="bk")
            bv_sb = const.tile([128, 1], FP32, tag="bv")
            nc.sync.dma_start(bq_sb, bq[:, None])
            nc.sync.dma_start(bk_sb, bk[:, None])
            nc.sync.dma_start(bv_sb, bv[:, None])
            npb = NKB - first_pad_kj  # 512-blocks that can contain padding
            if npb > 0:
                pb_sb = const.tile([128, B, npb * 512], FP32, tag="pb")
                nc.sync.dma_start(pb_sb, pb[:, :, first_pad_kj * 512:])

            # ---- persistent per-core intermediates ----
            # QT/KT: [2*HD qdims (h0 rows 0:64, h1 rows 64:128), B*S tokens]
            QT = big.tile([128, T], F32R, tag="QT")
            KT = big.tile([128, T], F32R, tag="KT")
            # V: [128 token-part, 64 token-chunks, 128 vdims(2 heads)]
            V = big.tile([128, T // 128, 128], F32R, tag="V")

            # ---- internal DRAM for the AllToAll ----
            with tc.tile_pool(name="dram", bufs=1, space="DRAM") as dpool:
                a2a_in = dpool.tile([NC_, 128, TPC], F32R, tag="a2a_in")
                a2a_out = dpool.tile([NC_, 128, TPC], F32R, tag="a2a_out")

                # ================= Phase B: QKV projection =================
                xT_r = xT.rearrange("(fo p) t -> p fo t", p=128)
                with tc.tile_pool(name="qkv", bufs=3) as qkvp, \
                     tc.tile_pool(name="qkv_ps", bufs=2, space="PSUM") as qps:
                    for tb in range(T // 512):
                        xt = qkvp.tile([128, 8, 512], F32R, tag="xt")
                        nc.sync.dma_start(xt, xT_r[:, :, tb * 512:(tb + 1) * 512])
                        psq = qps.tile([128, 512], FP32, tag="psq")
                        psk = qps.tile([128, 512], FP32, tag="psk")
                        psv = qps.tile([128, 512], FP32, tag="psv")
                        for fo in range(8):
                            nc.tensor.matmul(psq, wq_sb[:, fo], xt[:, fo],
                                             start=(fo == 0), stop=(fo == 7))
                        for fo in range(8):
                            nc.tensor.matmul(psk, wk_sb[:, fo], xt[:, fo],
                                             start=(fo == 0), stop=(fo == 7))
                        for fo in range(8):
                            nc.tensor.matmul(psv, wv_sb[:, fo], xt[:, fo],
                                             start=(fo == 0), stop=(fo == 7))
                        sl = slice(tb * 512, (tb + 1) * 512)
                        nc.scalar.activation(QT[:, sl], psq, IDENT, bias=bq_sb)
                        nc.scalar.activation(KT[:, sl], psk, IDENT, bias=bk_sb)
                        # V^T [vdim, tok] -> transpose to V [tok, vdim]
                        vt = qkvp.tile([128, 512], F32R, tag="vt")
                        nc.scalar.activation(vt, psv, IDENT, bias=bv_sb)
                        for t4 in range(4):
                            pst = qps.tile([128, 128], F32R, tag="pst")
                            nc.tensor.transpose(
                                pst, vt[:, t4 * 128:(t4 + 1) * 128], ident)
                            nc.scalar.copy(out=V[:, tb * 4 + t4, :], in_=pst)

                # ================= Phase C: attention =================
                with tc.tile_pool(name="att", bufs=2) as att, \
                     tc.tile_pool(name="attb", bufs=2) as attb, \
                     tc.tile_pool(name="P", bufs=1) as Pp, \
                     tc.tile_pool(name="pt", bufs=3) as ptp, \
                     tc.tile_pool(name="att_ps", bufs=2, space="PSUM") as aps:
                    for h in range(HPC):
                        hsl = slice(h * 64, h * 64 + 64)
                        for qmb in range(NKB):
                            nkb = qmb + 1
                            kw = nkb * 512  # key width
                            # bias rows for this (h, qmb), shared by 4 batches
                            bias_ts = []
                            for qi in range(4):
                                qt = qmb * 4 + qi
                                bt = attb.tile([128, NKB * 512], BF16,
                                               tag=f"bias{qi}")
                                nc.sync.dma_start(
                                    bt[:, :kw],
                                    biasc[h, qt * 128:(qt + 1) * 128, :kw])
                                bias_ts.append(bt)
                            for b in range(B):
                                Pb = Pp.tile([128, 4, NKB * 512], F32R, tag="P")
                                for qi in range(4):
                                    qt = qmb * 4 + qi
                                    s_sb = att.tile([128, NKB * 512], FP32,
                                                    tag="s")
                                    for kj in range(nkb):
                                        ps = aps.tile([128, 512], FP32,
                                                      tag="s_ps")
                                        nc.tensor.matmul(
                                            ps,
                                            QT[hsl, b * S + qt * 128:
                                               b * S + (qt + 1) * 128],
                                            KT[hsl, b * S + kj * 512:
                                               b * S + (kj + 1) * 512],
                                            start=True, stop=True)
                                        ksl = slice(kj * 512, (kj + 1) * 512)
                                        nc.vector.tensor_add(
                                            out=s_sb[:, ksl], in0=ps,
                                            in1=bias_ts[qi][:, ksl])
                                        if kj >= first_pad_kj:
                                            nc.vector.tensor_add(
                                                out=s_sb[:, ksl],
                                                in0=s_sb[:, ksl],
                                                in1=pb_sb[:, b,
                                                          (kj - first_pad_kj) * 512:
                                                          (kj - first_pad_kj + 1) * 512])
                                    rs = att.tile([128, 1], FP32, tag="rs")
                                    nc.scalar.activation(
                                        Pb[:, qi, :kw], s_sb[:, :kw], EXP,
                                        accum_out=rs)
                                    rc = att.tile([128, 1], FP32, tag="rc")
                                    nc.vector.reciprocal(rc, rs)
                                    nc.vector.tensor_scalar_mul(
                                        Pb[:, qi, :kw], Pb[:, qi, :kw], rc)
                                # transpose P; accumulate V^T @ P^T
                                av_ps = aps.tile([64, 512], FP32, tag="av")
                                for kc in range(nkb * 4):
                                    pt_ps = aps.tile([128, 512], F32R,
                                                     tag="ptps")
                                    for qi in range(4):
                                        nc.tensor.transpose(
                                            pt_ps[:, qi * 128:(qi + 1) * 128],
                                            Pb[:, qi, kc * 128:(kc + 1) * 128],
                                            ident)
                                    pt_sb = ptp.tile([128, 512], F32R,
                                                     tag="pt")
                                    nc.scalar.copy(out=pt_sb, in_=pt_ps)
                                    nc.tensor.matmul(
                                        av_ps, V[:, b * 16 + kc, hsl], pt_sb,
                                        start=(kc == 0),
                                        stop=(kc == nkb * 4 - 1))
                                av_sb = att.tile([64, 512], F32R, tag="avsb")
                                nc.scalar.copy(out=av_sb, in_=av_ps)
                                g = b * S + qmb * 512
                                nc.sync.dma_start(
                                    a2a_in[g // TPC, h * 64:h * 64 + 64,
                                           (g % TPC):(g % TPC) + 512],
                                    av_sb)

                # ============== Phase D: AllToAll + out-proj ==============
                nc.gpsimd.collective_compute(
                    "AllToAll", mybir.AluOpType.bypass,
                    replica_groups=[list(range(NC_))],
                    ins=[a2a_in[:]], outs=[a2a_out[:]])

                with tc.tile_pool(name="proj", bufs=1) as proj, \
                     tc.tile_pool(name="proj_w", bufs=2) as projw, \
                     tc.tile_pool(name="proj_ps", bufs=2, space="PSUM") as pps:
                    wout_sb = proj.tile([128, 8, D], F32R, tag="wout")
                    nc.sync.dma_start(
                        wout_sb, wout.rearrange("(io p) n -> p io n", p=128))
                    boutb_sb = proj.tile([128, D], FP32, tag="boutb")
                    nc.sync.dma_start(boutb_sb, boutb[:])
                    npad_sb = proj.tile([128, npads], FP32, tag="npad")
                    nc.sync.dma_start(npad_sb, npad[:])
                    recv = []
                    for i in range(NC_):
                        r = proj.tile([128, TPC], F32R, tag=f"recv{i}")
                        nc.sync.dma_start(r, a2a_out[i])
                        recv.append(r)
                    for tt in range(npads):
                        o_sb = projw.tile([128, D], FP32, tag="osb")
                        for nb in range(2):
                            ps = pps.tile([128, 512], FP32, tag="o_ps")
                            for i in range(NC_):
                                nc.tensor.matmul(
                                    ps, recv[i][:, tt * 128:(tt + 1) * 128],
                                    wout_sb[:, i, nb * 512:(nb + 1) * 512],
                                    start=(i == 0), stop=(i == NC_ - 1))
                            nsl = slice(nb * 512, (nb + 1) * 512)
                            nc.vector.tensor_add(
                                out=o_sb[:, nsl], in0=ps, in1=boutb_sb[:, nsl])
                        nc.vector.tensor_scalar_mul(
                            o_sb, o_sb, npad_sb[:, tt:tt + 1])
                        nc.sync.dma_start(
                            out[tt * 128:(tt + 1) * 128, :], o_sb)
    nc.finalize()
    return nc


_CACHE: dict = {}


def _prep_inputs(x, Wqkv, bqkv, Wout, bout, causal_mask, rel_bias,
                 key_padding_mask):
    """Host-side shard prep: returns (in_maps, first_pad_kj)."""
    f32 = np.float32
    bf16 = ml_dtypes.bfloat16
    x = np.asarray(x, f32)
    Wqkv = np.asarray(Wqkv, f32)
    bqkv = np.asarray(bqkv, f32)
    Wout = np.asarray(Wout, f32)
    bout = np.asarray(bout, f32)
    causal_mask = np.asarray(causal_mask, f32)
    rel_bias = np.asarray(rel_bias, f32)
    kpm = np.asarray(key_padding_mask, bool)

    scale = f32(HD ** -0.5)
    xT = np.ascontiguousarray(x.reshape(T, D).T)

    pad_cols = np.where(kpm.any(axis=0))[0]
    first_pad_kj = int(pad_cols.min() // 512) if pad_cols.size else NKB
    # additive key-padding bias, broadcast to 128 partitions: [128, B, S]
    pbm = np.where(kpm, f32(NEG), f32(0.0)).astype(f32)       # [B, S]
    pbm = np.ascontiguousarray(np.broadcast_to(pbm[None], (128, B, S)))
    boutb = np.ascontiguousarray(np.broadcast_to(bout[None], (128, D)))
    notpad_flat = (~kpm).reshape(T).astype(f32)

    in_maps = []
    for c in range(NC_):
        co = 128 * c
        wq_c = np.ascontiguousarray(Wqkv[:, co:co + 128] * scale)
        wk_c = np.ascontiguousarray(Wqkv[:, D + co:D + co + 128])
        wv_c = np.ascontiguousarray(Wqkv[:, 2 * D + co:2 * D + co + 128])
        bq_c = np.ascontiguousarray(bqkv[co:co + 128] * scale)
        bk_c = np.ascontiguousarray(bqkv[D + co:D + co + 128])
        bv_c = np.ascontiguousarray(bqkv[2 * D + co:2 * D + co + 128])
        bias_c = (rel_bias[HPC * c:HPC * c + HPC] + causal_mask[None]
                  ).astype(bf16)
        np_c = np.ascontiguousarray(
            notpad_flat[c * TPC:(c + 1) * TPC].reshape(TPC // 128, 128).T)
        in_maps.append({
            "xT": xT, "wq": wq_c, "wk": wk_c, "wv": wv_c,
            "bq": bq_c, "bk": bk_c, "bv": bv_c,
            "biasc": np.ascontiguousarray(bias_c), "pb": pbm,
            "wout": Wout, "boutb": boutb, "npad": np_c,
        })
    return in_maps, first_pad_kj


def kernel(**inputs) -> np.ndarray:
    in_maps, first_pad_kj = _prep_inputs(**inputs)
    key = ("prog", first_pad_kj)
    if key not in _CACHE:
        _CACHE[key] = build_program(first_pad_kj)
    nc = _CACHE[key]
    res = run_bass_kernel_spmd(nc, in_maps, core_ids=list(range(NC_)))
    outs = [res.results[c]["out"] for c in range(NC_)]
    return np.concatenate(outs, axis=0).reshape(B, S, D)


# revision 8
# speedup vs baseline: 1.9594x; 1.1879x over previous
"""Trainium2 Bass kernel for 16-head causal attention with relative position
bias (B=4, S=2048, D=1024, H=16, HD=64), distributed over 8 NeuronCores.

Sharding: tensor-parallel over heads — each core owns 2 heads end-to-end
(QKV projection column-sharded, attention, then an on-device AllToAll
re-shards by tokens so each core runs the output projection for a disjoint
1024-token slice). Host only slices weights / concatenates output slices.

Attention is computed in transposed orientation: scores^T [k, q] via
K @ Q^T, so the P^T needed by the P@V matmul is produced directly by the
exp() — no PE transposes of the probability matrix. The softmax row-sum is
obtained by appending a ones-column to V (row 64 of the attention matmul
accumulator), and 1/rowsum is broadcast with a rank-1 matmul and applied
while copying the accumulator out of PSUM. Key-padding enters through the
per-partition bias operand of the Exp activation (k is the partition dim).

Compute dtype: float32r (full-rate fp32 storage, ~1.5e-4 matmul rel err);
logits, exp and all accumulation stay fp32.

Host-side prep folds work into the data layout:
  - x passed transposed ([D, B*S]) to feed matmuls directly as rhs.
  - HD^-0.5 folded into Wq/bq.
  - causal mask folded into rel_bias, TRANSPOSED per head ([k, q]) and
    shipped as bf16 to halve DMA.
Softmax needs no max-subtraction: logits are O(6), exp is safe in fp32.
"""

import numpy as np
import ml_dtypes

import concourse.bass as bass
import concourse.mybir as mybir
from concourse import bacc
from concourse.tile import TileContext
from concourse.masks import make_identity
from concourse.bass_utils import run_bass_kernel_spmd

B, S, D, H = 4, 2048, 1024, 16
HD = D // H                  # 64
NC_ = 8                      # cores
HPC = H // NC_               # 2 heads per core
T = B * S                    # 8192 tokens
TPC = T // NC_               # 1024 tokens per core (out-proj shard)
NEG = -1e9
FP32 = mybir.dt.float32
F32R = mybir.dt.float32r
BF16 = mybir.dt.bfloat16

NKB = S // 512               # 4 k-blocks (and q macro blocks) per sequence
KTILES = S // 128            # 16 k-tiles per sequence
IDENT = mybir.ActivationFunctionType.Identity
EXP = mybir.ActivationFunctionType.Exp


def build_program() -> bass.Bass:
    """Build the (identical-on-every-core) SPMD Bass program."""
    nc = bacc.Bacc(num_devices=NC_)

    # ---- I/O ----
    xT = nc.dram_tensor("xT", [D, T], F32R, kind="ExternalInput")
    wq = nc.dram_tensor("wq", [D, 128], F32R, kind="ExternalInput")
    wk = nc.dram_tensor("wk", [D, 128], F32R, kind="ExternalInput")
    wv = nc.dram_tensor("wv", [D, 128], F32R, kind="ExternalInput")
    bq = nc.dram_tensor("bq", [128], FP32, kind="ExternalInput")
    bk = nc.dram_tensor("bk", [128], FP32, kind="ExternalInput")
    bv = nc.dram_tensor("bv", [128], FP32, kind="ExternalInput")
    # transposed bias: biasT[h, k, q] = rel_bias[h, q, k] + causal[q, k]
    biasT = nc.dram_tensor("biasT", [HPC, S, S], BF16, kind="ExternalInput")
    # key-padding additive column: pc[p, b, kc] = NEG if token kc*128+p padded
    pc = nc.dram_tensor("pc", [128, B, KTILES], FP32, kind="ExternalInput")
    wout = nc.dram_tensor("wout", [D, D], F32R, kind="ExternalInput")
    boutb = nc.dram_tensor("boutb", [128, D], FP32, kind="ExternalInput")
    npad = nc.dram_tensor("npad", [128, TPC // 128], FP32, kind="ExternalInput")
    out = nc.dram_tensor("out", [TPC, D], FP32, kind="ExternalOutput")

    npads = TPC // 128  # 8 token tiles in out-proj

    with TileContext(nc) as tc:
        with tc.tile_pool(name="const", bufs=1) as const, \
             tc.tile_pool(name="big", bufs=1) as big:
            # ---- constants ----
            ident_f32 = const.tile([128, 128], FP32, tag="ident_f32")
            make_identity(nc, ident_f32)
            ident = const.tile([128, 128], F32R, tag="ident")
            nc.vector.tensor_copy(out=ident, in_=ident_f32)
            ones_row = const.tile([1, 64], FP32, tag="ones")
            nc.vector.memset(ones_row, 1.0)
            wq_sb = const.tile([128, 8, 128], F32R, tag="wq")
            wk_sb = const.tile([128, 8, 128], F32R, tag="wk")
            wv_sb = const.tile([128, 8, 128], F32R, tag="wv")
            nc.sync.dma_start(wq_sb, wq.rearrange("(fo p) m -> p fo m", p=128))
            nc.sync.dma_start(wk_sb, wk.rearrange("(fo p) m -> p fo m", p=128))
            nc.sync.dma_start(wv_sb, wv.rearrange("(fo p) m -> p fo m", p=128))
            bq_sb = const.tile([128, 1], FP32, tag="bq")
            bk_sb = const.tile([128, 1], FP32, tag="bk")
            bv_sb = const.tile([128, 1], FP32, tag="bv")
            nc.sync.dma_start(bq_sb, bq[:, None])
            nc.sync.dma_start(bk_sb, bk[:, None])
            nc.sync.dma_start(bv_sb, bv[:, None])
            pc_sb = const.tile([128, B, KTILES], FP32, tag="pc")
            nc.sync.dma_start(pc_sb, pc[:])

            # ---- persistent per-core intermediates ----
            # QT/KT: [2*HD qdims (h0 rows 0:64, h1 rows 64:128), B*S tokens]
            QT = big.tile([128, T], F32R, tag="QT")
            KT = big.tile([128, T], F32R, tag="KT")
            # V': [128 token-part, 64 token-chunks, 130]:
            #   cols 0:64 head0, 64 ones, 65:129 head1, 129 ones
            V = big.tile([128, T // 128, 130], F32R, tag="V")
            nc.vector.memset(V[:, :, 64:65].bitcast(FP32), 1.0)
            nc.vector.memset(V[:, :, 129:130].bitcast(FP32), 1.0)

            # ---- internal DRAM for the AllToAll ----
            with tc.tile_pool(name="dram", bufs=1, space="DRAM") as dpool:
                a2a_in = dpool.tile([NC_, 128, TPC], F32R, tag="a2a_in")
                a2a_out = dpool.tile([NC_, 128, TPC], F32R, tag="a2a_out")

                # ================= Phase B: QKV projection =================
                xT_r = xT.rearrange("(fo p) t -> p fo t", p=128)
                with tc.tile_pool(name="qkv", bufs=3) as qkvp, \
                     tc.tile_pool(name="qkv_ps", bufs=2, space="PSUM") as qps:
                    for tb in range(T // 512):
                        xt = qkvp.tile([128, 8, 512], F32R, tag="xt")
                        nc.sync.dma_start(xt, xT_r[:, :, tb * 512:(tb + 1) * 512])
                        psq = qps.tile([128, 512], FP32, tag="psq")
                        psk = qps.tile([128, 512], FP32, tag="psk")
                        psv = qps.tile([128, 512], FP32, tag="psv")
                        for fo in range(8):
                            nc.tensor.matmul(psq, wq_sb[:, fo], xt[:, fo],
                                             start=(fo == 0), stop=(fo == 7))
                        for fo in range(8):
                            nc.tensor.matmul(psk, wk_sb[:, fo], xt[:, fo],
                                             start=(fo == 0), stop=(fo == 7))
                        for fo in range(8):
                            nc.tensor.matmul(psv, wv_sb[:, fo], xt[:, fo],
                                             start=(fo == 0), stop=(fo == 7))
                        sl = slice(tb * 512, (tb + 1) * 512)
                        nc.scalar.activation(QT[:, sl], psq, IDENT, bias=bq_sb)
                        nc.scalar.activation(KT[:, sl], psk, IDENT, bias=bk_sb)
                        # V^T [vdim, tok] -> transpose to V [tok, vdim]
                        vt = qkvp.tile([128, 512], F32R, tag="vt")
                        nc.scalar.activation(vt, psv, IDENT, bias=bv_sb)
                        for t4 in range(4):
                            pst = qps.tile([128, 128], F32R, tag="pst")
                            nc.tensor.transpose(
                                pst, vt[:, t4 * 128:(t4 + 1) * 128], ident)
                            c = tb * 4 + t4
                            nc.scalar.add(V[:, c, 0:64], pst[:, 0:64], 0.0)
                            nc.scalar.add(V[:, c, 65:129], pst[:, 64:128], 0.0)

                # ================= Phase C: attention =================
                with tc.tile_pool(name="att", bufs=3) as att, \
                     tc.tile_pool(name="attb", bufs=2) as attb, \
                     tc.tile_pool(name="pt", bufs=4) as ptp, \
                     tc.tile_pool(name="att_ps", bufs=3, space="PSUM") as aps, \
                     tc.tile_pool(name="av_ps", bufs=2, space="PSUM") as avps:
                    for h in range(HPC):
                        hsl = slice(h * 64, h * 64 + 64)
                        vsl = slice(h * 65, h * 65 + 65)
                        for qmb in range(NKB):
                            nkt = (qmb + 1) * 4  # k-tiles needed (causal)
                            # bias^T tiles for this (h, qmb): [k-tile, q-slice]
                            # cached across the 4 batches
                            bias_ts = []
                            for kc in range(nkt):
                                bt = attb.tile([128, 512], BF16,
                                               tag=f"bt{kc}")
                                nc.sync.dma_start(
                                    bt,
                                    biasT[h, kc * 128:(kc + 1) * 128,
                                          qmb * 512:(qmb + 1) * 512])
                                bias_ts.append(bt)
                            for b in range(B):
                                av = avps.tile([65, 512], FP32, tag="av")
                                qsl = slice(b * S + qmb * 512,
                                            b * S + (qmb + 1) * 512)
                                for kc in range(nkt):
                                    ps = aps.tile([128, 512], FP32, tag="s_ps")
                                    nc.tensor.matmul(
                                        ps,
                                        KT[hsl, b * S + kc * 128:
                                           b * S + (kc + 1) * 128],
                                        QT[hsl, qsl],
                                        start=True, stop=True)
                                    s_sb = att.tile([128, 512], FP32, tag="s")
                                    nc.vector.tensor_add(
                                        out=s_sb, in0=ps, in1=bias_ts[kc])
                                    pt = ptp.tile([128, 512], F32R, tag="pt")
                                    nc.scalar.activation(
                                        pt, s_sb, EXP,
                                        bias=pc_sb[:, b, kc:kc + 1])
                                    nc.tensor.matmul(
                                        av, V[:, b * 16 + kc, vsl], pt,
                                        start=(kc == 0), stop=(kc == nkt - 1))
                                # normalize by 1/rowsum (row 64 of av)
                                rc = att.tile([1, 512], FP32, tag="rc")
                                nc.vector.reciprocal(rc, av[64:65, :])
                                bc_ps = aps.tile([64, 512], FP32, tag="bc")
                                nc.tensor.matmul(bc_ps, ones_row, rc,
                                                 start=True, stop=True)
                                bc_sb = att.tile([64, 512], FP32, tag="bcs")
                                nc.vector.tensor_copy(out=bc_sb, in_=bc_ps)
                                av_sb = att.tile([64, 512], F32R, tag="avsb")
                                nc.vector.tensor_tensor(
                                    out=av_sb, in0=av[0:64, :], in1=bc_sb,
                                    op=mybir.AluOpType.mult)
                                g = b * S + qmb * 512
                                nc.sync.dma_start(
                                    a2a_in[g // TPC, h * 64:h * 64 + 64,
                                           (g % TPC):(g % TPC) + 512],
                                    av_sb)

                # ============== Phase D: AllToAll + out-proj ==============
                nc.gpsimd.collective_compute(
                    "AllToAll", mybir.AluOpType.bypass,
                    replica_groups=[list(range(NC_))],
                    ins=[a2a_in[:]], outs=[a2a_out[:]])

                with tc.tile_pool(name="proj", bufs=1) as proj, \
                     tc.tile_pool(name="proj_w", bufs=2) as projw, \
                     tc.tile_pool(name="proj_ps", bufs=2, space="PSUM") as pps:
                    wout_sb = proj.tile([128, 8, D], F32R, tag="wout")
                    nc.sync.dma_start(
                        wout_sb, wout.rearrange("(io p) n -> p io n", p=128))
                    boutb_sb = proj.tile([128, D], FP32, tag="boutb")
                    nc.sync.dma_start(boutb_sb, boutb[:])
                    npad_sb = proj.tile([128, npads], FP32, tag="npad")
                    nc.sync.dma_start(npad_sb, npad[:])
                    recv = []
                    for i in range(NC_):
                        r = proj.tile([128, TPC], F32R, tag=f"recv{i}")
                        nc.sync.dma_start(r, a2a_out[i])
                        recv.append(r)
                    for tt in range(npads):
                        o_sb = projw.tile([128, D], FP32, tag="osb")
                        for nb in range(2):
                            ps = pps.tile([128, 512], FP32, tag="o_ps")
                            for i in range(NC_):
                                nc.tensor.matmul(
                                    ps, recv[i][:, tt * 128:(tt + 1) * 128],
                                    wout_sb[:, i, nb * 512:(nb + 1) * 512],
                                    start=(i == 0), stop=(i == NC_ - 1))
                            nsl = slice(nb * 512, (nb + 1) * 512)
                            nc.vector.tensor_add(
                                out=o_sb[:, nsl], in0=ps, in1=boutb_sb[:, nsl])
                        nc.vector.tensor_scalar_mul(
                            o_sb, o_sb, npad_sb[:, tt:tt + 1])
                        nc.sync.dma_start(
                            out[tt * 128:(tt + 1) * 128, :], o_sb)
    nc.finalize()
    return nc


_CACHE: dict = {}


def _prep_inputs(x, Wqkv, bqkv, Wout, bout, causal_mask, rel_bias,
                 key_padding_mask):
    """Host-side shard prep: returns in_maps."""
    f32 = np.float32
    bf16 = ml_dtypes.bfloat16
    x = np.asarray(x, f32)
    Wqkv = np.asarray(Wqkv, f32)
    bqkv = np.asarray(bqkv, f32)
    Wout = np.asarray(Wout, f32)
    bout = np.asarray(bout, f32)
    causal_mask = np.asarray(causal_mask, f32)
    rel_bias = np.asarray(rel_bias, f32)
    kpm = np.asarray(key_padding_mask, bool)

    scale = f32(HD ** -0.5)
    xT = np.ascontiguousarray(x.reshape(T, D).T)

    # key-padding additive column per k-tile: [128, B, KTILES]
    pcm = np.where(kpm, f32(NEG), f32(0.0)).astype(f32)       # [B, S]
    pcm = np.ascontiguousarray(
        pcm.reshape(B, KTILES, 128).transpose(2, 0, 1))       # [128, B, KT]
    boutb = np.ascontiguousarray(np.broadcast_to(bout[None], (128, D)))
    notpad_flat = (~kpm).reshape(T).astype(f32)

    in_maps = []
    for c in range(NC_):
        co = 128 * c
        wq_c = np.ascontiguousarray(Wqkv[:, co:co + 128] * scale)
        wk_c = np.ascontiguousarray(Wqkv[:, D + co:D + co + 128])
        wv_c = np.ascontiguousarray(Wqkv[:, 2 * D + co:2 * D + co + 128])
        bq_c = np.ascontiguousarray(bqkv[co:co + 128] * scale)
        bk_c = np.ascontiguousarray(bqkv[D + co:D + co + 128])
        bv_c = np.ascontiguousarray(bqkv[2 * D + co:2 * D + co + 128])
        bias_c = rel_bias[HPC * c:HPC * c + HPC] + causal_mask[None]
        biasT_c = np.ascontiguousarray(
            bias_c.transpose(0, 2, 1).astype(bf16))
        np_c = np.ascontiguousarray(
            notpad_flat[c * TPC:(c + 1) * TPC].reshape(TPC // 128, 128).T)
        in_maps.append({
            "xT": xT, "wq": wq_c, "wk": wk_c, "wv": wv_c,
            "bq": bq_c, "bk": bk_c, "bv": bv_c,
            "biasT": biasT_c, "pc": pcm,
            "wout": Wout, "boutb": boutb, "npad": np_c,
        })
    return in_maps


def kernel(**inputs) -> np.ndarray:
    in_maps = _prep_inputs(**inputs)
    if "prog" not in _CACHE:
        _CACHE["prog"] = build_program()
    nc = _CACHE["prog"]
    res = run_bass_kernel_spmd(nc, in_maps, core_ids=list(range(NC_)))
    outs = [res.results[c]["out"] for c in range(NC_)]
    return np.concatenate(outs, axis=0).reshape(B, S, D)
